# revision 40
# baseline (speedup 1.0000x reference)
"""Trainium2 Bass kernel for the shared-weight multi-head attention problem.

Math (per batch b, head h, with x_h = x[b,:,h*64:(h+1)*64] [S, d]):
    q = k = x_h @ W + b
    s = q @ q^T / d               (symmetric!)
    t = s + (1-mask_q) * (-1e6)   (constant per softmax row -> only effect is
                                   the fp32 quantization of s for masked rows)
    out_h = softmax(t) @ x_h

Device strategy (8 cores, data parallel over (batch, head-group-of-8)):
  - F orientation: score tiles are [k partitions, q free]; softmax columns
    are queries. No max-subtraction (scores are small); shift-invariance
    makes this exact to fp rounding.
  - Queries are pre-sorted per batch (unmasked first) on the host, so the
    mask is block-constant outside one [win_lo, win_hi) window.
  - Masked-query fp32 quantization is reproduced exactly by a -C extra
    contraction row (C = 3*2^24, ulp 4 == the reference's 0.0625 lattice
    after /64), restored by +C (DVE in the mixed window, fused ACT bias in
    the fully-masked range).
  - Bias b is folded into the projection matmul as an extra contraction row
    (xt row 64 = ones, W65 row 64 = b), so q tiles are plain PSUM->SBUF
    copies.
  - For fully-unmasked q blocks the scores moving operand is ql itself
    (tail row ones x ones adds +1 to every score of the block: a per-column
    constant that cancels exactly in softmax). qr (tail row = -C*(1-m)) is
    produced by the otherwise-idle GPSIMD engine only for blocks that touch
    masked queries.
  - ACT (exp) is the bottleneck engine: scores and PV matmuls are emitted
    interleaved with la=1 so the 3 PSUM score slots keep ACT fed during PV.
  - The output stays in F orientation [65, q] (row 64 = softmax denominator
    from the ones column of the PV stationary); normalize + transpose
    happen on the host during unsharding.
"""

import numpy as np

B, S, D, H, d = 4, 2048, 1024, 16, 64
NH = 8          # heads per core
NCORES = 8
# Mask shift constant in the pre-/64 score domain. Any C with C/4 an even
# integer and C±|s| inside one fp32 binade of ulp 4 reproduces the reference's
# quantization lattice (0.0625 after /64) including tie behavior; 3*2^24 is
# additionally exact in bf16, so every operand path carries it exactly.
CMASK = np.float32(3 * 2**24)  # 50331648

_NC_CACHE = {}


def _build_nc(s=S, nh=NH, win_lo=0, win_hi=0, reps=1, la=1):
    import concourse.bacc as bacc
    import concourse.tile as tile
    from concourse import mybir
    from concourse.masks import make_identity

    f32 = mybir.dt.float32
    f32r = mybir.dt.float32r
    bf16 = mybir.dt.bfloat16
    Exp = mybir.ActivationFunctionType.Exp

    kt = s // 128          # k tiles per head (16)
    nj = s // 512          # q column blocks per head (4)
    CH = float(CMASK)

    wn = win_hi - win_lo

    nc = bacc.Bacc("TRN2", target_bir_lowering=False, debug=False)

    x_in = nc.declare_dram_parameter("x", [s, nh * d], f32, isOutput=False)
    w_in = nc.declare_dram_parameter("W", [d, d], f32, isOutput=False)
    b_in = nc.declare_dram_parameter("b", [1, d], f32, isOutput=False)
    c_in = nc.declare_dram_parameter("crows", [2, s], f32, isOutput=False)
    out = nc.declare_dram_parameter("out", [nh, nj, 65, 512], f32, isOutput=True)

    with tile.TileContext(nc) as tc:
        with (
            tc.tile_pool(name="const", bufs=1) as const_pool,
            tc.tile_pool(name="xh", bufs=1) as xh_pool,
            tc.tile_pool(name="xhr", bufs=2) as xhr_pool,
            tc.tile_pool(name="f", bufs=8 * 4 + 2) as f_pool,
            tc.tile_pool(name="at", bufs=2) as at_pool,
            tc.tile_pool(name="ps", bufs=3, space="PSUM") as ps_pool,
            tc.tile_pool(name="po", bufs=1, space="PSUM") as po_pool,
            tc.tile_pool(name="pm", bufs=1, space="PSUM") as pm_pool,
        ):
            # ---- one-time constants -----------------------------------
            ident = const_pool.tile([128, 128], f32, tag="ident")
            make_identity(nc, ident[:])
            ident_bf = const_pool.tile([128, 128], bf16, tag="identbf")
            nc.vector.tensor_copy(ident_bf[:], ident[:])

            # W65: rows 0:64 = W, row 64 = b (bias via extra contraction row)
            w_raw = const_pool.tile([65, d], f32, tag="wraw")
            nc.sync.dma_start(w_raw[0:64, :], w_in[:, :])
            nc.sync.dma_start(w_raw[64:65, :], b_in[:, :])
            w65 = const_pool.tile([65, d], f32r, tag="w65")
            nc.vector.tensor_copy(w65[:], w_raw[:])

            # per-partition +C/64 bias for the fully-masked exp restore
            biasq = const_pool.tile([128, 1], f32, tag="biasq")
            nc.vector.memset(biasq[:], CH / 64.0)

            # straddling q block (window end mid-block): its fully-masked
            # tail gets +C via an accumulating 1-row matmul (ones x cpos)
            jfix = win_hi // 512 if (0 < win_hi < nj * 512 and win_hi % 512) else None
            fixw = (jfix + 1) * 512 - win_hi if jfix is not None else 0
            ones_bf = cpos_bf = None
            if jfix is not None:
                ones_bf = const_pool.tile([1, 128], bf16, tag="onesbf")
                cpos_bf = const_pool.tile([1, fixw], bf16, tag="cposbf")

            # +C*(1-m) over [win_lo, win_hi) broadcast to all partitions and
            # doubled, so a [128, 2, wn] view restores both 512-halves of a
            # ps tile in one DVE op. Built by two stride-0-source DMAs.
            cposB2 = None
            if wn > 0:
                cposB2 = const_pool.tile([128, 2 * wn], f32, tag="cposB2")

            crow_neg = const_pool.tile([1, s], f32, tag="cneg")

            def emit_cpos_dmas():
                """Issued on the ACT hwdge queue: idle at startup, keeps the
                SP queue clear for the x loads."""
                nc.scalar.dma_start(crow_neg[:], c_in[0:1, :])
                if wn > 0:
                    src = c_in[1:2, win_lo:win_hi].partition_broadcast(64)
                    for p in range(0, 128, 64):
                        cv = cposB2[p : p + 64, :].rearrange(
                            "p (u c) -> p u c", u=2
                        )
                        nc.scalar.dma_start(cv[:, :, :], src.broadcast_to((64, 2, wn)))
                if jfix is not None:
                    crow_pos = const_pool.tile([1, fixw], f32, tag="cpos")
                    nc.scalar.dma_start(crow_pos[:], c_in[1:2, win_hi : win_hi + fixw])
                    nc.gpsimd.memset(ones_bf[:], 1.0)
                    nc.gpsimd.tensor_copy(cpos_bf[:], crow_pos[:])

            # ---- persistent per-parity tiles --------------------------
            # ql: rows 0:64 = q^T (bf16), row 64 = ones
            # qr: rows 0:64 = q^T,        row 64 = -C*(1-m)
            # xt: rows 0:64 = x_h^T (f32r), row 64 = ones
            ones_f32 = const_pool.tile([1, s], f32, tag="ones_f32")
            nc.vector.memset(ones_f32[:], 1.0)
            qls, qrs, xts = [], [], []
            for p in range(2):
                ql = const_pool.tile([65, s], bf16, tag=f"ql{p}")
                qls.append(ql)
                qr = const_pool.tile([65, s], bf16, tag=f"qr{p}")
                qrs.append(qr)
                xt = const_pool.tile([65, s], f32r, tag=f"xt{p}")
                xts.append(xt)

            def emit_slot_init(p):
                """Tail rows of the parity-p persistent tiles (once); all on
                DVE -- Pool must stay clear for the xhb/qr block copies."""
                nc.vector.memset(qls[p][64:65, :], 1.0)
                nc.vector.tensor_copy(qrs[p][64:65, :], crow_neg[0:1, :])
                nc.vector.tensor_copy(xts[p][64:65, :], ones_f32[:])

            xh = [None] * nh

            def emit_load(h):
                """DMA x for head h into SBUF, ones-interleaved [x_t | 1].

                4 strided DMAs (4 k-tiles each) instead of 16: SP dispatch is
                500ns per instruction and x loads would otherwise clog the
                queue ahead of latency-sensitive small DMAs.
                """
                t_ = xh_pool.tile([128, kt * 65], f32, tag=f"xh{h}")
                xh[h] = t_
                tv = t_[:].rearrange("p (t c) -> p t c", t=kt)
                for t0 in range(0, kt, 4):
                    nc.sync.dma_start(
                        tv[:, t0 : t0 + 4, 0:64],
                        x_in[
                            t0 * 128 : (t0 + 4) * 128, h * 64 : (h + 1) * 64
                        ].rearrange("(t p) c -> p t c", p=128),
                    )
                nc.vector.memset(t_[:, 64 : kt * 65 : 65], 1.0)

            def emit_head_prep(h):
                """Loads + transposes + projection for head h."""
                if h + 2 < nh:
                    emit_load(h + 2)

                # bf16 copy of [x_h | 1]: PV stationary + transpose source
                # (GPSIMD so DVE stays free for the J loop; chunked so chunk
                # j only waits on its own 4 x DMAs)
                xhb = xhr_pool.tile([128, kt * 65], bf16, tag="xhb")

                ql, qr, xt = qls[h % 2], qrs[h % 2], xts[h % 2]
                # interleave transposes with projection so scores can start
                # as soon as ql columns 0:512 exist
                for j in range(nj):
                    c0, c1 = 4 * j * 65, (4 * j + 4) * 65
                    nc.gpsimd.tensor_copy(xhb[:, c0:c1], xh[h][:, c0:c1])
                    for t in range(4 * j, 4 * j + 4):
                        pt = pm_pool.tile([d, 128], bf16, tag="pm")
                        nc.tensor.transpose(
                            pt[:], xhb[:, t * 65 : t * 65 + 64], ident_bf[:, :]
                        )
                        nc.vector.tensor_copy(xt[0:64, t * 128 : (t + 1) * 128], pt[:])
                    pq = pm_pool.tile([d, 512], f32, tag="pm")
                    nc.tensor.matmul(
                        pq[:], w65[:], xt[:, j * 512 : (j + 1) * 512],
                        start=True, stop=True,
                    )
                    nc.vector.tensor_copy(ql[0:64, j * 512 : (j + 1) * 512], pq[:])
                    # per-block so block j of the next head's scores only
                    # waits on its own projection
                    nc.gpsimd.tensor_copy(
                            qr[0:64, j * 512 : (j + 1) * 512],
                            ql[0:64, j * 512 : (j + 1) * 512],
                        )
                return xhb, ql, qr

            # ---- symmetric exp mirroring ------------------------------
            # F = bf16(exp(...)) is symmetric wherever both the k-tile and
            # the q-block sit on the same (uniform) side of the mask window:
            # quantization applies to both or neither, and the mirrored
            # values are bit-exact (same contraction order, same restores).
            # Such pieces are produced by DMA-XBAR transposes of the already
            # exp'd source block -- zero PE/ACT/DVE cost.
            jorder = [1, 0, 2, 3] if nj == 4 else list(range(nj))
            jpos = {J: r for r, J in enumerate(jorder)}

            def _side(lo, hi):
                if hi <= win_lo:
                    return "u"
                if lo >= win_hi:
                    return "m"
                return None

            def mirror_src(a, J):
                """Source q block J' if (k-tile a, q block J) can be mirrored."""
                Jsrc = a // 4
                if jpos[Jsrc] >= jpos[J]:
                    return None
                sa = _side(a * 128, (a + 1) * 128)
                if sa is None or sa != _side(J * 512, (J + 1) * 512):
                    return None
                return Jsrc

            def emit_scores_tile(h, ql, qr, J, i, fs_of):
                """PSUM tile i (k-tiles 2i,2i+1) of q block J -> exp'd F tile."""
                z0, z2 = J * 512, (J + 1) * 512
                rr0, rr1 = max(z0, win_lo), min(z2, win_hi)
                # straddling block: +C for the fully-masked tail [m0, z2) is
                # applied as a third accumulating matmul (same fp32 rounding
                # as a post-add), so the ACT bias stays uniform per tile
                m0 = max(z0, win_hi)
                mm_fix = z0 < win_hi < z2
                ft = f_pool.tile([128, 1024], bf16, tag="F")
                mir = [mirror_src(2 * i, J), mirror_src(2 * i + 1, J)]
                for u in range(2):
                    if mir[u] is None:
                        continue
                    off = (2 * i + u) * 128 - mir[u] * 512
                    fsrc = fs_of[mir[u]]
                    for m in range(4):
                        tq = 4 * J + m
                        nc.sync.dma_start(
                            ft[:, u * 512 + m * 128 : u * 512 + (m + 1) * 128],
                            fsrc[tq // 2][
                                :, (tq % 2) * 512 + off : (tq % 2) * 512 + off + 128
                            ],
                            transpose=True,
                        )
                live = [u for u in range(2) if mir[u] is None]
                if not live:
                    return ft
                ps = ps_pool.tile([128, 1024], f32, tag="ps")
                for u in live:
                    a = 2 * i + u
                    nc.tensor.matmul(
                        ps[:, u * 512 : (u + 1) * 512],
                        ql[:, a * 128 : (a + 1) * 128],
                        qr[:, z0:z2],
                        start=True, stop=not mm_fix,
                    )
                    if mm_fix:
                        nc.tensor.matmul(
                            ps[:, u * 512 + (m0 - z0) : (u + 1) * 512],
                            ones_bf[:, 0:128],
                            cpos_bf[:, 0 : z2 - m0],
                            start=False, stop=True,
                        )
                if rr0 < rr1:
                    # +C restore, both 512-halves in one op via doubled
                    # cposB2 (window blocks are never mirrored: len(live)==2)
                    c0 = rr0 - z0
                    w = rr1 - rr0
                    ps3 = ps[:].rearrange("p (u c) -> p u c", u=2)
                    cb = cposB2[:, 0 : 2 * wn].rearrange("p (u c) -> p u c", u=2)
                    nc.vector.tensor_add(
                        ps3[:, :, c0 : c0 + w],
                        ps3[:, :, c0 : c0 + w],
                        cb[:, :, rr0 - win_lo : rr1 - win_lo],
                    )
                bias = biasq[:] if z0 >= win_hi else 0.0
                if len(live) == 2:
                    nc.scalar.activation(ft[:], ps[:], Exp, bias=bias, scale=1.0 / 64.0)
                else:
                    u = live[0]
                    nc.scalar.activation(
                        ft[:, u * 512 : (u + 1) * 512],
                        ps[:, u * 512 : (u + 1) * 512],
                        Exp, bias=bias, scale=1.0 / 64.0,
                    )
                return ft

            def emit_pv_group(ctx, i, po):
                hp, xhbp, fs = ctx
                for t in (2 * i, 2 * i + 1):
                    nc.tensor.matmul(
                        po[:],
                        xhbp[:, t * 65 : t * 65 + 65],
                        fs[t // 2][:, (t % 2) * 512 : (t % 2 + 1) * 512],
                        start=(t == 0),
                        stop=(t == kt - 1),
                    )

            def emit_pv_finish(ctx, J, po):
                hp = ctx[0]
                at = at_pool.tile([65, 512], f32, tag="at")
                nc.vector.tensor_copy(at[:], po[:])
                nc.sync.dma_start(out[hp, J, :, :], at[:])

            def emit_head_jloop(h, xhb, ql, qr, carry):
                """Scores+exp for head h, interleaved with the la-behind PV.
                The first round drains `carry` -- the previous head's last
                block -- so PE never runs a PV tail back-to-back."""
                fs_of = {}
                for r, J in enumerate(jorder):
                    fs_of[J] = []
                    if r >= la:
                        Jp = jorder[r - la]
                        pctx = (h, xhb, fs_of[Jp])
                    elif carry is not None:
                        Jp, pctx = carry[r]
                    else:
                        Jp = pctx = None
                    if Jp is not None:
                        po = po_pool.tile([65, 512], f32, tag="po")
                    for i in range(kt // 2):
                        fs_of[J].append(emit_scores_tile(h, ql, qr, J, i, fs_of))
                        if Jp is not None:
                            emit_pv_group(pctx, i, po)
                    if Jp is not None:
                        emit_pv_finish(pctx, Jp, po)
                return [(Jp, (h, xhb, fs_of[Jp])) for Jp in jorder[nj - la :]]

            def emit_flush(carry):
                for Jp, pctx in carry:
                    po = po_pool.tile([65, 512], f32, tag="po")
                    for i in range(kt // 2):
                        emit_pv_group(pctx, i, po)
                    emit_pv_finish(pctx, Jp, po)

            # ---- head pipeline ---------------------------------------
            carry = None
            for _rep in range(reps):
                emit_load(0)
                if nh > 1:
                    emit_load(1)
                preps = {}
                preps[0] = emit_head_prep(0)
                if _rep == 0:
                    # tail-row inits and window DMAs: emitted after prep(0)
                    # (they fill engine idle during the first x loads)
                    emit_cpos_dmas()
                    emit_slot_init(0)
                if nh > 1:
                    preps[1] = emit_head_prep(1)
                if _rep == 0:
                    emit_slot_init(1)
                for h in range(nh):
                    carry = emit_head_jloop(h, *preps.pop(h), carry)
                    if h + 2 < nh:
                        preps[h + 2] = emit_head_prep(h + 2)
            emit_flush(carry)

    nc.compile()
    return nc


def get_nc(s=S, nh=NH, win_lo=0, win_hi=0, reps=1, la=1):
    key = (s, nh, win_lo, win_hi, reps, la)
    if key not in _NC_CACHE:
        _NC_CACHE[key] = _build_nc(s, nh, win_lo, win_hi, reps, la)
    return _NC_CACHE[key]


def plan_mask(mask):
    """Per-batch query permutation (unmasked first) + global mixed window."""
    mask = np.asarray(mask)
    orders = [np.argsort(-mask[bb], kind="stable") for bb in range(mask.shape[0])]
    n1s = [int(mask[bb].sum()) for bb in range(mask.shape[0])]
    lo, hi = min(n1s), max(n1s)
    win_lo = (lo // 128) * 128
    win_hi = -(-hi // 128) * 128
    return orders, win_lo, win_hi


def make_in_maps(x, mask, W, b, orders, s=S, nh=NH):
    """Shard full inputs into per-core input maps (core = batch*2 + head_group)."""
    x = np.asarray(x, dtype=np.float32)
    mask = np.asarray(mask)
    W = np.ascontiguousarray(np.asarray(W, dtype=np.float32))
    bv = np.ascontiguousarray(np.asarray(b, dtype=np.float32).reshape(1, d))
    in_maps = []
    for c in range(NCORES):
        bb, hg = c // 2, c % 2
        order = orders[bb]
        xs = np.ascontiguousarray(x[bb][order, hg * nh * d : (hg + 1) * nh * d])
        m1 = np.float32(1.0) - mask[bb][order].astype(np.float32)
        cneg = (-CMASK * m1).astype(np.float32)
        crows = np.ascontiguousarray(np.stack([cneg, -cneg], axis=0))
        in_maps.append({"x": xs, "W": W, "b": bv, "crows": crows})
    return in_maps


def gather_out(results, orders):
    """results: list of 8 dicts with 'out' [NH, 4, 65, 512] -> full [B, S, D].

    Host-side finish: softmax normalization (row 64 = denominators) and the
    F-orientation -> [q, d] transpose, then the query-permutation scatter.
    """
    a = np.empty((B, H, S, d), np.float32)
    for c in range(NCORES):
        bb, hg = c // 2, c % 2
        o = np.asarray(results[c]["out"])          # [NH, nj, 65, 512]
        num = o[:, :, 0:64, :]                     # [NH, nj, 64, 512]
        den = o[:, :, 64:65, :]                    # [NH, nj, 1, 512]
        # -> [NH, q, d] with q = nj*512
        res = (num / den).transpose(0, 1, 3, 2).reshape(NH, S, d)
        a[bb, hg * NH : (hg + 1) * NH][:, orders[bb], :] = res
    return a.reshape(B, S, D)


def kernel(x, mask, W, b):
    from concourse.bass_utils import run_bass_kernel_spmd

    orders, win_lo, win_hi = plan_mask(mask)
    nc = get_nc(win_lo=win_lo, win_hi=win_hi)
    in_maps = make_in_maps(x, mask, W, b, orders)
    res = run_bass_kernel_spmd(nc, in_maps, list(range(NCORES)))
    return gather_out(res.results, orders)


# revision 48
# speedup vs baseline: 2.6283x; 2.6283x over previous
"""Trainium2 Bass kernel for the shared-weight multi-head attention problem.

Math (per batch b, head h, with x_h = x[b,:,h*64:(h+1)*64] [S, d]):
    q = k = x_h @ W + b
    s = q @ q^T / d               (symmetric!)
    t = s + (1-mask_q) * (-1e6)   (constant per softmax row -> only effect is
                                   the fp32 quantization of s for masked rows)
    out_h = softmax(t) @ x_h

Device strategy (8 cores, data parallel over (batch, head-group-of-8)):
  - F orientation: score tiles are [k partitions, q free]; softmax columns
    are queries. No max-subtraction (scores are small); shift-invariance
    makes this exact to fp rounding.
  - Queries are pre-sorted per batch (unmasked first) on the host, so the
    mask is block-constant outside one [win_lo, win_hi) window.
  - Masked-query fp32 quantization is reproduced exactly by a -C extra
    contraction row (C = 3*2^24, ulp 4 == the reference's 0.0625 lattice
    after /64), restored by +C (DVE in the mixed window, fused ACT bias in
    the fully-masked range).
  - Bias b is folded into the projection matmul as an extra contraction row
    (xt row 64 = ones, W65 row 64 = b), so q tiles are plain PSUM->SBUF
    copies.
  - The scores moving operand qr (tail row = -C*(1-m), data rows copied
    from ql by the otherwise-idle GPSIMD engine) keeps every q block on the
    same additive path, so symmetric F pieces are bit-identical.
  - ACT (exp) is the bottleneck engine: scores and PV matmuls are emitted
    interleaved with la=1 so the 3 PSUM score slots keep ACT fed during PV;
    the la-behind PV block carries across head boundaries.
  - Where the k-tile and q-block sit on the same uniform side of the mask
    window, F = bf16(exp(.)) is symmetric and mirror pieces are produced by
    transposing already-exp'd tiles (mir= dma: DMA-XBAR; pe: PE transpose +
    DVE copy; off: disabled), skipping those exps entirely.
  - The output stays in F orientation [65, q] (row 64 = softmax denominator
    from the ones column of the PV stationary); normalize + transpose
    happen on the host during unsharding.
"""

import numpy as np

B, S, D, H, d = 4, 2048, 1024, 16, 64
NH = 8          # heads per core
NCORES = 8
# Mask shift constant in the pre-/64 score domain. Any C with C/4 an even
# integer and C±|s| inside one fp32 binade of ulp 4 reproduces the reference's
# quantization lattice (0.0625 after /64) including tie behavior; 3*2^24 is
# additionally exact in bf16, so every operand path carries it exactly.
CMASK = np.float32(3 * 2**24)  # 50331648

_NC_CACHE = {}


def _build_nc(s=S, nh=NH, win_lo=0, win_hi=0, reps=1, la=1, mir="dma"):
    import concourse.bacc as bacc
    import concourse.tile as tile
    from concourse import mybir
    from concourse.masks import make_identity

    f32 = mybir.dt.float32
    f32r = mybir.dt.float32r
    bf16 = mybir.dt.bfloat16
    Exp = mybir.ActivationFunctionType.Exp

    kt = s // 128          # k tiles per head (16)
    nj = s // 512          # q column blocks per head (4)
    CH = float(CMASK)

    wn = win_hi - win_lo

    nc = bacc.Bacc("TRN2", target_bir_lowering=False, debug=False)

    x_in = nc.declare_dram_parameter("x", [s, nh * d], f32, isOutput=False)
    w_in = nc.declare_dram_parameter("W", [d, d], f32, isOutput=False)
    b_in = nc.declare_dram_parameter("b", [1, d], f32, isOutput=False)
    c_in = nc.declare_dram_parameter("crows", [2, s], f32, isOutput=False)
    out = nc.declare_dram_parameter("out", [nh, nj, 65, 512], f32, isOutput=True)

    with tile.TileContext(nc) as tc:
        with (
            tc.tile_pool(name="const", bufs=1) as const_pool,
            tc.tile_pool(name="xh", bufs=1) as xh_pool,
            tc.tile_pool(name="xhr", bufs=2) as xhr_pool,
            tc.tile_pool(name="f", bufs=8 * 4 + 2) as f_pool,
            tc.tile_pool(name="at", bufs=2) as at_pool,
            tc.tile_pool(name="ps", bufs=3, space="PSUM") as ps_pool,
            tc.tile_pool(name="po", bufs=1, space="PSUM") as po_pool,
            tc.tile_pool(name="pm", bufs=1, space="PSUM") as pm_pool,
        ):
            # ---- one-time constants -----------------------------------
            ident = const_pool.tile([128, 128], f32, tag="ident")
            make_identity(nc, ident[:])
            ident_bf = const_pool.tile([128, 128], bf16, tag="identbf")
            nc.vector.tensor_copy(ident_bf[:], ident[:])

            # W65: rows 0:64 = W, row 64 = b (bias via extra contraction row)
            w_raw = const_pool.tile([65, d], f32, tag="wraw")
            nc.sync.dma_start(w_raw[0:64, :], w_in[:, :])
            nc.sync.dma_start(w_raw[64:65, :], b_in[:, :])
            w65 = const_pool.tile([65, d], f32r, tag="w65")
            nc.vector.tensor_copy(w65[:], w_raw[:])

            # per-partition +C/64 bias for the fully-masked exp restore
            biasq = const_pool.tile([128, 1], f32, tag="biasq")
            nc.vector.memset(biasq[:], CH / 64.0)

            # straddling q block (window end mid-block): its fully-masked
            # tail gets +C via an accumulating 1-row matmul (ones x cpos)
            jfix = win_hi // 512 if (0 < win_hi < nj * 512 and win_hi % 512) else None
            fixw = (jfix + 1) * 512 - win_hi if jfix is not None else 0
            ones_bf = cpos_bf = None
            if jfix is not None:
                ones_bf = const_pool.tile([1, 128], bf16, tag="onesbf")
                cpos_bf = const_pool.tile([1, fixw], bf16, tag="cposbf")

            # +C*(1-m) over [win_lo, win_hi) broadcast to all partitions and
            # doubled, so a [128, 2, wn] view restores both 512-halves of a
            # ps tile in one DVE op. Built by two stride-0-source DMAs.
            cposB2 = None
            if wn > 0:
                cposB2 = const_pool.tile([128, 2 * wn], f32, tag="cposB2")

            crow_neg = const_pool.tile([1, s], f32, tag="cneg")

            def emit_cpos_dmas():
                """Issued on the ACT hwdge queue: idle at startup, keeps the
                SP queue clear for the x loads."""
                nc.scalar.dma_start(crow_neg[:], c_in[0:1, :])
                if wn > 0:
                    src = c_in[1:2, win_lo:win_hi].partition_broadcast(64)
                    for p in range(0, 128, 64):
                        cv = cposB2[p : p + 64, :].rearrange(
                            "p (u c) -> p u c", u=2
                        )
                        nc.scalar.dma_start(cv[:, :, :], src.broadcast_to((64, 2, wn)))
                if jfix is not None:
                    crow_pos = const_pool.tile([1, fixw], f32, tag="cpos")
                    nc.scalar.dma_start(crow_pos[:], c_in[1:2, win_hi : win_hi + fixw])
                    nc.gpsimd.memset(ones_bf[:], 1.0)
                    nc.gpsimd.tensor_copy(cpos_bf[:], crow_pos[:])

            # ---- persistent per-parity tiles --------------------------
            # ql: rows 0:64 = q^T (bf16), row 64 = ones
            # qr: rows 0:64 = q^T,        row 64 = -C*(1-m)
            # xt: rows 0:64 = x_h^T (f32r), row 64 = ones
            ones_f32 = const_pool.tile([1, s], f32, tag="ones_f32")
            nc.vector.memset(ones_f32[:], 1.0)
            qls, qrs, xts = [], [], []
            for p in range(2):
                ql = const_pool.tile([65, s], bf16, tag=f"ql{p}")
                qls.append(ql)
                qr = const_pool.tile([65, s], bf16, tag=f"qr{p}")
                qrs.append(qr)
                xt = const_pool.tile([65, s], f32r, tag=f"xt{p}")
                xts.append(xt)

            def emit_slot_init(p):
                """Tail rows of the parity-p persistent tiles (once); all on
                DVE -- Pool must stay clear for the xhb/qr block copies."""
                nc.vector.memset(qls[p][64:65, :], 1.0)
                nc.vector.tensor_copy(qrs[p][64:65, :], crow_neg[0:1, :])
                nc.vector.tensor_copy(xts[p][64:65, :], ones_f32[:])

            xh = [None] * nh

            def emit_load(h):
                """DMA x for head h into SBUF, ones-interleaved [x_t | 1].

                4 strided DMAs (4 k-tiles each) instead of 16: SP dispatch is
                500ns per instruction and x loads would otherwise clog the
                queue ahead of latency-sensitive small DMAs.
                """
                t_ = xh_pool.tile([128, kt * 65], f32, tag=f"xh{h}")
                xh[h] = t_
                tv = t_[:].rearrange("p (t c) -> p t c", t=kt)
                for t0 in range(0, kt, 4):
                    nc.sync.dma_start(
                        tv[:, t0 : t0 + 4, 0:64],
                        x_in[
                            t0 * 128 : (t0 + 4) * 128, h * 64 : (h + 1) * 64
                        ].rearrange("(t p) c -> p t c", p=128),
                    )
                nc.vector.memset(t_[:, 64 : kt * 65 : 65], 1.0)

            def emit_head_prep(h):
                """Loads + transposes + projection for head h."""
                if h + 2 < nh:
                    emit_load(h + 2)

                # bf16 copy of [x_h | 1]: PV stationary + transpose source
                # (GPSIMD so DVE stays free for the J loop; chunked so chunk
                # j only waits on its own 4 x DMAs)
                xhb = xhr_pool.tile([128, kt * 65], bf16, tag="xhb")

                ql, qr, xt = qls[h % 2], qrs[h % 2], xts[h % 2]
                # interleave transposes with projection so scores can start
                # as soon as ql columns 0:512 exist
                for j in range(nj):
                    c0, c1 = 4 * j * 65, (4 * j + 4) * 65
                    nc.gpsimd.tensor_copy(xhb[:, c0:c1], xh[h][:, c0:c1])
                    for t in range(4 * j, 4 * j + 4):
                        pt = pm_pool.tile([d, 128], bf16, tag="pm")
                        nc.tensor.transpose(
                            pt[:], xhb[:, t * 65 : t * 65 + 64], ident_bf[:, :]
                        )
                        nc.vector.tensor_copy(xt[0:64, t * 128 : (t + 1) * 128], pt[:])
                    pq = pm_pool.tile([d, 512], f32, tag="pm")
                    nc.tensor.matmul(
                        pq[:], w65[:], xt[:, j * 512 : (j + 1) * 512],
                        start=True, stop=True,
                    )
                    nc.vector.tensor_copy(ql[0:64, j * 512 : (j + 1) * 512], pq[:])
                    # per-block so block j of the next head's scores only
                    # waits on its own projection
                    nc.gpsimd.tensor_copy(
                            qr[0:64, j * 512 : (j + 1) * 512],
                            ql[0:64, j * 512 : (j + 1) * 512],
                        )
                return xhb, ql, qr

            # ---- symmetric exp mirroring ------------------------------
            # F = bf16(exp(...)) is symmetric wherever both the k-tile and
            # the q-block sit on the same (uniform) side of the mask window:
            # quantization applies to both or neither, and the mirrored
            # values are bit-exact (same contraction order, same restores).
            # Such pieces are produced by DMA-XBAR transposes of the already
            # exp'd source block -- zero PE/ACT/DVE cost.
            jorder = [1, 0, 2, 3] if nj == 4 else list(range(nj))
            jpos = {J: r for r, J in enumerate(jorder)}

            def _side(lo, hi):
                if hi <= win_lo:
                    return "u"
                if lo >= win_hi:
                    return "m"
                return None

            def mirror_src(a, J):
                """Source q block J' if (k-tile a, q block J) can be mirrored."""
                if mir == "off":
                    return None
                Jsrc = a // 4
                if jpos[Jsrc] >= jpos[J]:
                    return None
                sa = _side(a * 128, (a + 1) * 128)
                if sa is None or sa != _side(J * 512, (J + 1) * 512):
                    return None
                return Jsrc

            def emit_scores_tile(h, ql, qr, J, i, fs_of):
                """PSUM tile i (k-tiles 2i,2i+1) of q block J -> exp'd F tile."""
                z0, z2 = J * 512, (J + 1) * 512
                rr0, rr1 = max(z0, win_lo), min(z2, win_hi)
                # straddling block: +C for the fully-masked tail [m0, z2) is
                # applied as a third accumulating matmul (same fp32 rounding
                # as a post-add), so the ACT bias stays uniform per tile
                m0 = max(z0, win_hi)
                mm_fix = z0 < win_hi < z2
                ft = f_pool.tile([128, 1024], bf16, tag="F")
                mirs = [mirror_src(2 * i, J), mirror_src(2 * i + 1, J)]
                for u in range(2):
                    if mirs[u] is None:
                        continue
                    off = (2 * i + u) * 128 - mirs[u] * 512
                    fsrc = fs_of[mirs[u]]
                    if mir == "pe":
                        pt2 = ps_pool.tile([128, 1024], bf16, tag="ps")
                    for m in range(4):
                        tq = 4 * J + m
                        sl = fsrc[tq // 2][
                            :, (tq % 2) * 512 + off : (tq % 2) * 512 + off + 128
                        ]
                        if mir == "pe":
                            nc.tensor.transpose(
                                pt2[:, m * 128 : (m + 1) * 128], sl, ident_bf[:, :]
                            )
                        else:
                            nc.sync.dma_start(
                                ft[:, u * 512 + m * 128 : u * 512 + (m + 1) * 128],
                                sl,
                                transpose=True,
                            )
                    if mir == "pe":
                        nc.vector.tensor_copy(
                            ft[:, u * 512 : (u + 1) * 512], pt2[:, 0:512]
                        )
                live = [u for u in range(2) if mirs[u] is None]
                if not live:
                    return ft
                ps = ps_pool.tile([128, 1024], f32, tag="ps")
                for u in live:
                    a = 2 * i + u
                    nc.tensor.matmul(
                        ps[:, u * 512 : (u + 1) * 512],
                        ql[:, a * 128 : (a + 1) * 128],
                        qr[:, z0:z2],
                        start=True, stop=not mm_fix,
                    )
                    if mm_fix:
                        nc.tensor.matmul(
                            ps[:, u * 512 + (m0 - z0) : (u + 1) * 512],
                            ones_bf[:, 0:128],
                            cpos_bf[:, 0 : z2 - m0],
                            start=False, stop=True,
                        )
                if rr0 < rr1:
                    # +C restore, both 512-halves in one op via doubled
                    # cposB2 (window blocks are never mirrored: len(live)==2)
                    c0 = rr0 - z0
                    w = rr1 - rr0
                    ps3 = ps[:].rearrange("p (u c) -> p u c", u=2)
                    cb = cposB2[:, 0 : 2 * wn].rearrange("p (u c) -> p u c", u=2)
                    nc.vector.tensor_add(
                        ps3[:, :, c0 : c0 + w],
                        ps3[:, :, c0 : c0 + w],
                        cb[:, :, rr0 - win_lo : rr1 - win_lo],
                    )
                bias = biasq[:] if z0 >= win_hi else 0.0
                if len(live) == 2:
                    nc.scalar.activation(ft[:], ps[:], Exp, bias=bias, scale=1.0 / 64.0)
                else:
                    u = live[0]
                    nc.scalar.activation(
                        ft[:, u * 512 : (u + 1) * 512],
                        ps[:, u * 512 : (u + 1) * 512],
                        Exp, bias=bias, scale=1.0 / 64.0,
                    )
                return ft

            def emit_pv_group(ctx, i, po):
                hp, xhbp, fs = ctx
                for t in (2 * i, 2 * i + 1):
                    nc.tensor.matmul(
                        po[:],
                        xhbp[:, t * 65 : t * 65 + 65],
                        fs[t // 2][:, (t % 2) * 512 : (t % 2 + 1) * 512],
                        start=(t == 0),
                        stop=(t == kt - 1),
                    )

            def emit_pv_finish(ctx, J, po):
                hp = ctx[0]
                at = at_pool.tile([65, 512], f32, tag="at")
                nc.vector.tensor_copy(at[:], po[:])
                nc.sync.dma_start(out[hp, J, :, :], at[:])

            def emit_head_jloop(h, xhb, ql, qr, carry):
                """Scores+exp for head h, interleaved with the la-behind PV.
                The first round drains `carry` -- the previous head's last
                block -- so PE never runs a PV tail back-to-back."""
                fs_of = {}
                for r, J in enumerate(jorder):
                    fs_of[J] = []
                    if r >= la:
                        Jp = jorder[r - la]
                        pctx = (h, xhb, fs_of[Jp])
                    elif carry is not None:
                        Jp, pctx = carry[r]
                    else:
                        Jp = pctx = None
                    if Jp is not None:
                        po = po_pool.tile([65, 512], f32, tag="po")
                    for i in range(kt // 2):
                        fs_of[J].append(emit_scores_tile(h, ql, qr, J, i, fs_of))
                        if Jp is not None:
                            emit_pv_group(pctx, i, po)
                    if Jp is not None:
                        emit_pv_finish(pctx, Jp, po)
                return [(Jp, (h, xhb, fs_of[Jp])) for Jp in jorder[nj - la :]]

            def emit_flush(carry):
                for Jp, pctx in carry:
                    po = po_pool.tile([65, 512], f32, tag="po")
                    for i in range(kt // 2):
                        emit_pv_group(pctx, i, po)
                    emit_pv_finish(pctx, Jp, po)

            # ---- head pipeline ---------------------------------------
            carry = None
            for _rep in range(reps):
                emit_load(0)
                if nh > 1:
                    emit_load(1)
                preps = {}
                preps[0] = emit_head_prep(0)
                if _rep == 0:
                    # tail-row inits and window DMAs: emitted after prep(0)
                    # (they fill engine idle during the first x loads)
                    emit_cpos_dmas()
                    emit_slot_init(0)
                if nh > 1:
                    preps[1] = emit_head_prep(1)
                if _rep == 0:
                    emit_slot_init(1)
                for h in range(nh):
                    carry = emit_head_jloop(h, *preps.pop(h), carry)
                    if h + 2 < nh:
                        preps[h + 2] = emit_head_prep(h + 2)
            emit_flush(carry)

    nc.compile()
    return nc


def get_nc(s=S, nh=NH, win_lo=0, win_hi=0, reps=1, la=1, mir="dma"):
    key = (s, nh, win_lo, win_hi, reps, la, mir)
    if key not in _NC_CACHE:
        _NC_CACHE[key] = _build_nc(s, nh, win_lo, win_hi, reps, la, mir)
    return _NC_CACHE[key]


def plan_mask(mask):
    """Per-batch query permutation (unmasked first) + global mixed window."""
    mask = np.asarray(mask)
    orders = [np.argsort(-mask[bb], kind="stable") for bb in range(mask.shape[0])]
    n1s = [int(mask[bb].sum()) for bb in range(mask.shape[0])]
    lo, hi = min(n1s), max(n1s)
    win_lo = (lo // 128) * 128
    win_hi = -(-hi // 128) * 128
    return orders, win_lo, win_hi


def make_in_maps(x, mask, W, b, orders, s=S, nh=NH):
    """Shard full inputs into per-core input maps (core = batch*2 + head_group)."""
    x = np.asarray(x, dtype=np.float32)
    mask = np.asarray(mask)
    W = np.ascontiguousarray(np.asarray(W, dtype=np.float32))
    bv = np.ascontiguousarray(np.asarray(b, dtype=np.float32).reshape(1, d))
    in_maps = []
    for c in range(NCORES):
        bb, hg = c // 2, c % 2
        order = orders[bb]
        xs = np.ascontiguousarray(x[bb][order, hg * nh * d : (hg + 1) * nh * d])
        m1 = np.float32(1.0) - mask[bb][order].astype(np.float32)
        cneg = (-CMASK * m1).astype(np.float32)
        crows = np.ascontiguousarray(np.stack([cneg, -cneg], axis=0))
        in_maps.append({"x": xs, "W": W, "b": bv, "crows": crows})
    return in_maps


def gather_out(results, orders):
    """results: list of 8 dicts with 'out' [NH, 4, 65, 512] -> full [B, S, D].

    Host-side finish: softmax normalization (row 64 = denominators) and the
    F-orientation -> [q, d] transpose, then the query-permutation scatter.
    """
    a = np.empty((B, H, S, d), np.float32)
    for c in range(NCORES):
        bb, hg = c // 2, c % 2
        o = np.asarray(results[c]["out"])          # [NH, nj, 65, 512]
        num = o[:, :, 0:64, :]                     # [NH, nj, 64, 512]
        den = o[:, :, 64:65, :]                    # [NH, nj, 1, 512]
        # -> [NH, q, d] with q = nj*512
        res = (num / den).transpose(0, 1, 3, 2).reshape(NH, S, d)
        a[bb, hg * NH : (hg + 1) * NH][:, orders[bb], :] = res
    return a.reshape(B, S, D)


def kernel(x, mask, W, b):
    from concourse.bass_utils import run_bass_kernel_spmd

    orders, win_lo, win_hi = plan_mask(mask)
    nc = get_nc(win_lo=win_lo, win_hi=win_hi, mir="pe")
    in_maps = make_in_maps(x, mask, W, b, orders)
    res = run_bass_kernel_spmd(nc, in_maps, list(range(NCORES)))
    return gather_out(res.results, orders)
